# revision 1
# baseline (speedup 1.0000x reference)
"""Trainium2 Bass kernel for nn_Grapher (EdgeConv GNN message passing).

Per image (one per NeuronCore): KNN over M=4096 nodes (C=96, K=9 incl. self),
EdgeConv MLP, mean-aggregate, ReLU.

Algorithm (restructured, numerically validated vs reference):
  - score s[m,n] = 2*x_m.x_n - |x_n|^2  (row-constant shift of -dist; same top-k)
    computed via one augmented matmul: L=[2x;1] (97,M) x R=[x;-sq] (97,N).
  - self (d=0) is always a neighbor -> suppress diagonal, take top-8 others
    with vector.max/max_index (ties -> lowest index, matching jax top_k).
  - EdgeConv MLP decomposes per-node: W1=[W1a;W1b],
      edge (i,j): h1 = LReLU(a_i + v_j),  a = x@(W1a-W1b)+b1, v = x@W1b
    and mean/W2 commute:  out_i = ReLU((1/9 * sum_k h1_k) @ W2 + b2).
  - v gathered by neighbor index via gpsimd dma_gather from a padded DRAM table.
"""
import sys

sys.path.insert(0, "/opt/trn_rl_repo")

import numpy as np

import concourse.bacc as bacc
import concourse.bass as bass
import concourse.tile as tile
from concourse import mybir
from concourse.bass_utils import run_bass_kernel_spmd

F32 = mybir.dt.float32
I16 = mybir.dt.int16
U16 = mybir.dt.uint16

B, C, H, W = 8, 96, 64, 64
N = H * W          # 4096 nodes per image
NT = N // 128      # 32 node tiles
K1 = C + 1         # augmented contraction dim
SLOPE = 0.01
BIG = 1e30


def build_program(repeat=1):
    nc = bacc.Bacc("TRN2", target_bir_lowering=False, debug=False)

    x_d = nc.dram_tensor("x", [C, N], F32, kind="ExternalInput")
    w1_d = nc.dram_tensor("W1", [2 * C, C], F32, kind="ExternalInput")
    b1_d = nc.dram_tensor("b1", [C], F32, kind="ExternalInput")
    w2_d = nc.dram_tensor("W2", [C, C], F32, kind="ExternalInput")
    b2_d = nc.dram_tensor("b2", [C], F32, kind="ExternalInput")
    out_d = nc.dram_tensor("out", [C, N], F32, kind="ExternalOutput")
    vpad_d = nc.dram_tensor("vpad", [N, 128], F32)        # gather table (padded rows)
    idxb_d = nc.dram_tensor("idxb", [N, 8], I16)          # neighbor idx, node-major
    idxw_d = nc.dram_tensor("idxw", [NT, 1024], I16)      # wrapped neighbor idx per tile

    with tile.TileContext(nc) as tc:
        with (
            tc.tile_pool(name="big", bufs=1) as bigp,
            tc.tile_pool(name="wts", bufs=1) as wp,
            tc.tile_pool(name="wk", bufs=3) as wk,
        ):
            # ---------------- constants / weights ----------------
            w1a = wp.tile([C, C], F32)
            w1b = wp.tile([C, C], F32)
            w2c = wp.tile([C, C], F32)
            b2pp = wp.tile([C, 1], F32)
            b1bc = wp.tile([128, C], F32)
            nc.sync.dma_start(w1a[:], w1_d[0:C, :])
            nc.sync.dma_start(w1b[:], w1_d[C:2 * C, :])
            nc.sync.dma_start(w2c[:], w2_d[:])
            nc.sync.dma_start(b2pp[:], bass.AP(b2_d, 0, [[1, C], [1, 1]]))
            # broadcast b1 across 128 partitions (step-0 DRAM re-read)
            nc.sync.dma_start(b1bc[:], bass.AP(b1_d, 0, [[0, 128], [1, C]]))
            wd = wp.tile([C, C], F32)
            nc.vector.tensor_sub(wd[:], w1a[:], w1b[:])

            ones96 = wp.tile([C, 1], F32)
            nc.vector.memset(ones96[:], 1.0)
            zeros128 = wp.tile([128, 128], F32)
            nc.vector.memset(zeros128[:], 0.0)
            diagbig = wp.tile([128, 128], F32)
            nc.gpsimd.affine_select(
                out=diagbig[:], in_=zeros128[:], pattern=[[1, 128]],
                compare_op=mybir.AluOpType.not_equal, fill=BIG,
                base=0, channel_multiplier=-1,
            )
            ident = wp.tile([128, 128], F32)
            nc.gpsimd.affine_select(
                out=ident[:], in_=zeros128[:], pattern=[[1, 128]],
                compare_op=mybir.AluOpType.not_equal, fill=1.0,
                base=0, channel_multiplier=-1,
            )

            # ---------------- load x, build L/R ----------------
            xt = bigp.tile([C, N], F32)
            nc.sync.dma_start(xt[:], x_d[:])

            L = bigp.tile([K1, N], F32)
            R = bigp.tile([K1, N], F32)
            nc.scalar.mul(L[0:C, :], xt[:], 2.0)      # funnels the x DMA too
            nc.vector.memset(L[C:K1, :], 1.0)
            nc.scalar.copy(R[0:C, :], xt[:])

            xsq = bigp.tile([C, N], F32)
            nc.vector.tensor_mul(xsq[:], xt[:], xt[:])
            v_sb = bigp.tile([128, NT, 128], F32)
            a_sb = bigp.tile([128, NT, C], F32)
            nc.vector.memset(v_sb[:, :, C:128], 0.0)
            with tc.tile_pool(name="psP", bufs=2, space="PSUM") as ps:
                for j in range(8):
                    sq_ps = ps.tile([1, 512], F32, tag="sq")
                    nc.tensor.matmul(sq_ps[:], lhsT=ones96[:], rhs=xsq[:, j * 512:(j + 1) * 512],
                                     start=True, stop=True)
                    nc.scalar.mul(R[C:K1, j * 512:(j + 1) * 512], sq_ps[:], -1.0)

                # ---------------- per-node a, v ----------------
                for t in range(NT):
                    tl = slice(t * 128, (t + 1) * 128)
                    v_ps = ps.tile([128, C], F32, tag="va")
                    nc.tensor.matmul(v_ps[:], lhsT=L[0:C, tl], rhs=w1b[:], start=True, stop=True)
                    # L rows 0:C hold 2x -> v computed with 2x needs scale 0.5
                    nc.scalar.mul(v_sb[:, t, 0:C], v_ps[:], 0.5)
                    a_ps = ps.tile([128, C], F32, tag="va")
                    nc.tensor.matmul(a_ps[:], lhsT=L[0:C, tl], rhs=wd[:], start=True, stop=True)
                    # a = 0.5*(2x)@wd + b1 : scalar_tensor_tensor (a_ps*0.5) + b1bc
                    nc.vector.scalar_tensor_tensor(
                        out=a_sb[:, t, :], in0=a_ps[:], scalar=0.5, in1=b1bc[:],
                        op0=mybir.AluOpType.mult, op1=mybir.AluOpType.add,
                    )
            nc.sync.dma_start(
                bass.AP(vpad_d, 0, [[128, 128], [128 * 128, NT], [1, 128]]),
                v_sb[:],
            )

            for rep in range(repeat):
                # ---------------- pass A: scores + top-8 ----------------
                s_sb = bigp.tile([128, N], F32)
                idx_all = bigp.tile([128, NT, 8], U16)
                with tc.tile_pool(name=f"psA{rep}", bufs=2, space="PSUM") as ps:
                  for t in range(NT):
                    tl = slice(t * 128, (t + 1) * 128)
                    for half in range(2):
                        s_ps = ps.tile([128, 2048], F32, tag="s")
                        for j in range(4):
                            nc.tensor.matmul(
                                s_ps[:, j * 512:(j + 1) * 512],
                                lhsT=L[:, tl],
                                rhs=R[:, half * 2048 + j * 512: half * 2048 + (j + 1) * 512],
                                start=True, stop=True,
                            )
                        nc.scalar.copy(s_sb[:, half * 2048:(half + 1) * 2048], s_ps[:])
                    nc.vector.tensor_sub(s_sb[:, tl], s_sb[:, tl], diagbig[:])
                    top8 = wk.tile([128, 8], F32, tag="top8")
                    nc.vector.max(out=top8[:], in_=s_sb[:])
                    nc.vector.max_index(out=idx_all[:, t, :], in_max=top8[:], in_values=s_sb[:])
                    nc.sync.dma_start(
                        idxb_d[t * 128:(t + 1) * 128, :],
                        idx_all[:, t, :].bitcast(I16),
                    )

                # ---------------- pass B: gather + MLP + reduce ----------------
                osb = bigp.tile([C, N], F32)
                with tc.tile_pool(name=f"psB{rep}", bufs=2, space="PSUM") as ps:
                  for t in range(NT):
                    # build wrapped idx for dma_gather: list[j] = idx[node j%128, slot j//128]
                    # wrapped[p16, s*8+nhi] = idxb[nhi*16+p16, s]; (s,nhi) transpose done on DVE
                    tmp1 = wk.tile([16, 64], I16, tag="tmp1")   # [p16, nhi*8+s]
                    nc.sync.dma_start(
                        tmp1[:].rearrange("p (n s) -> p n s", n=8),
                        bass.AP(idxb_d, t * 1024, [[8, 16], [128, 8], [1, 8]]),
                    )
                    tmp2 = wk.tile([16, 64], I16, tag="tmp2")   # [p16, s*8+nhi]
                    nc.vector.tensor_copy(
                        tmp2[:].rearrange("p (s n) -> p s n", s=8),
                        tmp1[:].rearrange("p (n s) -> p s n", n=8),
                    )
                    nc.sync.dma_start(
                        bass.AP(idxw_d, t * 1024, [[64, 16], [1, 64]]), tmp2[:],
                    )
                    widx = wk.tile([128, 64], I16, tag="widx")
                    for g in range(8):
                        nc.sync.dma_start(
                            widx[g * 16:(g + 1) * 16, :],
                            bass.AP(idxw_d, t * 1024, [[64, 16], [1, 64]]),
                        )
                    vg = wk.tile([128, 9, 128], F32, tag="vg")
                    nc.gpsimd.dma_gather(
                        out_ap=vg[:, 0:8, :], in_ap=vpad_d[:], idxs_ap=widx[:],
                        num_idxs=1024, num_idxs_reg=1024, elem_size=128,
                    )
                    nc.scalar.copy(vg[:, 8, 0:C], v_sb[:, t, 0:C])
                    zl = wk.tile([128, 9, C], F32, tag="zl")
                    vg_ap, a_bc = bass.broadcast_tensor_aps(
                        vg[:, :, 0:C], a_sb[:, t, :].rearrange("p (o c) -> p o c", o=1))
                    nc.vector.tensor_add(zl[:], vg_ap, a_bc)
                    nc.vector.scalar_tensor_tensor(
                        out=zl[:], in0=zl[:], scalar=SLOPE, in1=zl[:],
                        op0=mybir.AluOpType.mult, op1=mybir.AluOpType.max,
                    )
                    zs = wk.tile([128, C], F32, tag="zs")
                    nc.vector.tensor_reduce(
                        out=zs[:], in_=zl[:].rearrange("p s c -> p c s"),
                        axis=mybir.AxisListType.X, op=mybir.AluOpType.add,
                    )
                    zt_ps = ps.tile([C, 128], F32, tag="zt")
                    nc.tensor.transpose(zt_ps[:], zs[:], ident[:])
                    zst = wk.tile([C, 128], F32, tag="zst")
                    nc.scalar.copy(zst[:], zt_ps[:])
                    o_ps = ps.tile([C, 128], F32, tag="o")
                    nc.tensor.matmul(o_ps[:], lhsT=w2c[:], rhs=zst[:], start=True, stop=True)
                    nc.scalar.activation(
                        osb[:, t * 128:(t + 1) * 128], o_ps[:],
                        mybir.ActivationFunctionType.Relu, bias=b2pp[:], scale=1.0 / 9.0,
                    )
            nc.sync.dma_start(out_d[:], osb[:])
    nc.compile()
    return nc


_prog = None


def kernel(x, W1, b1, W2, b2):
    global _prog
    x = np.ascontiguousarray(np.asarray(x, dtype=np.float32))
    W1 = np.ascontiguousarray(np.asarray(W1, dtype=np.float32))
    b1 = np.ascontiguousarray(np.asarray(b1, dtype=np.float32))
    W2 = np.ascontiguousarray(np.asarray(W2, dtype=np.float32))
    b2 = np.ascontiguousarray(np.asarray(b2, dtype=np.float32))
    assert x.shape == (B, C, H, W)
    if _prog is None:
        _prog = build_program()
    xf = x.reshape(B, C, N)
    in_maps = [
        {"x": xf[b], "W1": W1, "b1": b1, "W2": W2, "b2": b2} for b in range(B)
    ]
    res = run_bass_kernel_spmd(_prog, in_maps, core_ids=list(range(B)))
    out = np.stack([res.results[b]["out"] for b in range(B)], 0)
    return out.reshape(B, C, H, W).astype(np.float32)


if __name__ == "__main__":
    rng = np.random.default_rng(0)
    ins = {
        "x": rng.standard_normal((B, C, H, W), dtype=np.float32),
        "W1": rng.standard_normal((2 * C, C), dtype=np.float32) * 0.07,
        "b1": rng.standard_normal((C,), dtype=np.float32) * 0.01,
        "W2": rng.standard_normal((C, C), dtype=np.float32) * 0.1,
        "b2": rng.standard_normal((C,), dtype=np.float32) * 0.01,
    }
    o = kernel(**ins)
    print("kernel ran, out shape", o.shape, "finite:", np.isfinite(o).all())

